# revision 8
# baseline (speedup 1.0000x reference)
"""DeepseekV2 MLA attention prefill kernel for 8 Trainium2 NeuronCores.

Sharding: 2-way data-parallel over batch x 4-way tensor-parallel over heads
(4 heads per core).  The shared projections (q down-proj + RMSNorm, compressed
KV + RoPE key) are computed on an S/4 slice per core inside each batch group
and exchanged with an in-group AllGather.  Per-head up-projections, attention
and the output projection are computed locally; the o_proj partial sums are
reduced on the host during unsharding.

Layouts: activations are feature-major ([D, S]) throughout, attention scores
are computed transposed ([s_k, s_q]) so the PV matmul needs no transposes.
RoPE is applied via host-side permuted/sign-folded weight columns, so the
device only does two elementwise multiplies and an add per rope tensor.
Matmuls run in bf16 (full PE rate) with fp32 PSUM accumulation; all
normalizations / softmax denominators are fp32.
"""
import sys
sys.path.insert(0, "/opt/trn_rl_repo")

import math
import numpy as np
import ml_dtypes

import concourse.bass as bass
import concourse.tile as tile
from concourse import bacc, mybir
from concourse.bass_utils import run_bass_kernel_spmd

# ---- problem constants (hardcoded; kernel.py must be self-contained) ----
B, S, HID, H = 2, 2048, 2048, 16
Q_LORA, KV_LORA = 1536, 512
D_NOPE, D_ROPE, D_V = 128, 64, 128
D_Q = D_NOPE + D_ROPE
EPS = 1e-6
ROPE_THETA = 10000.0
N_CORES = 8
HPC = 4                      # heads per core
GROUPS = [[0, 1, 2, 3], [4, 5, 6, 7]]

PLAN_B = True                # S-split shared compute + AllGather
S_LOC = S // 4 if PLAN_B else S
G_ROWS = Q_LORA + KV_LORA + D_ROPE   # 2112 rows in the gather payload

F32 = mybir.dt.float32
BF16 = mybir.dt.bfloat16
MM_DT = BF16                 # matmul input dtype

SCALE = 1.0 / math.sqrt(D_Q)

_CACHE = {}


# ---------------------------------------------------------------- builder --
def build_kernel(plan_b=PLAN_B, mm_dt=MM_DT):
    s_loc = S // 4 if plan_b else S
    nch_n = s_loc // 512

    nc = bacc.Bacc("TRN2", target_bir_lowering=False, debug=False,
                   num_devices=N_CORES)

    # inputs (bf16 weights/activations prepared on host)
    xt = nc.dram_tensor("xt", [HID, s_loc], mm_dt, kind="ExternalInput")
    wdq = nc.dram_tensor("wdq", [HID, Q_LORA], mm_dt, kind="ExternalInput")
    wuq = nc.dram_tensor("wuq", [Q_LORA, HPC * 256], mm_dt, kind="ExternalInput")
    wkva = nc.dram_tensor("wkva", [HID, KV_LORA + 2 * D_ROPE], mm_dt, kind="ExternalInput")
    wkvb = nc.dram_tensor("wkvb", [KV_LORA, HPC, 256], mm_dt, kind="ExternalInput")
    ow = nc.dram_tensor("ow", [HPC, D_V, HID], mm_dt, kind="ExternalInput")
    cos_l = nc.dram_tensor("cos_l", [D_ROPE, s_loc], F32, kind="ExternalInput")
    sin_l = nc.dram_tensor("sin_l", [D_ROPE, s_loc], F32, kind="ExternalInput")
    cos_f = nc.dram_tensor("cos_f", [D_ROPE, S], mm_dt, kind="ExternalInput")
    sin_f = nc.dram_tensor("sin_f", [D_ROPE, S], mm_dt, kind="ExternalInput")
    masks = nc.dram_tensor("masks", [4, 128, 512], mm_dt, kind="ExternalInput")
    out_t = nc.dram_tensor("out_t", [HID, S], F32, kind="ExternalOutput")

    with tile.TileContext(nc) as tc:
        import contextlib
        ctx = contextlib.ExitStack()
        with ctx:
            persist = ctx.enter_context(tc.tile_pool(name="persist", bufs=1))
            wpool = ctx.enter_context(tc.tile_pool(name="wpool", bufs=3))
            spool = ctx.enter_context(tc.tile_pool(name="spool", bufs=2))
            xpool = ctx.enter_context(tc.tile_pool(name="xpool", bufs=3))
            ppool = ctx.enter_context(tc.tile_pool(name="ppool", bufs=4, space="PSUM"))
            pscore = ctx.enter_context(tc.tile_pool(name="pscore", bufs=2, space="PSUM"))
            pctx = ctx.enter_context(tc.tile_pool(name="pctx", bufs=1, space="PSUM"))
            psums = ctx.enter_context(tc.tile_pool(name="psums", bufs=1, space="PSUM"))
            dram = ctx.enter_context(tc.tile_pool(name="dram", bufs=1, space="DRAM"))

            # ---- constants ----
            ones_sb = persist.tile([128, 1], mm_dt, tag="ones")
            nc.vector.memset(ones_sb, 1.0)
            eps_sb = persist.tile([1, 1], F32, tag="eps")
            nc.vector.memset(eps_sb, EPS)
            mask_sb = persist.tile([128, 4, 512], mm_dt, tag="masks")
            nc.sync.dma_start(out=mask_sb, in_=masks.ap().rearrange("d p c -> p d c"))
            cosl_sb = persist.tile([D_ROPE, s_loc], F32, tag="cosl")
            sinl_sb = persist.tile([D_ROPE, s_loc], F32, tag="sinl")
            nc.sync.dma_start(out=cosl_sb, in_=cos_l.ap())
            nc.sync.dma_start(out=sinl_sb, in_=sin_l.ap())
            cosf_sb = persist.tile([D_ROPE, 4, 512], mm_dt, tag="cosf")
            sinf_sb = persist.tile([D_ROPE, 4, 512], mm_dt, tag="sinf")
            nc.sync.dma_start(out=cosf_sb, in_=cos_f.ap().rearrange("d (c n) -> d c n", c=4))
            nc.sync.dma_start(out=sinf_sb, in_=sin_f.ap().rearrange("d (c n) -> d c n", c=4))

            # gather buffers (DRAM)
            g_in = dram.tile([G_ROWS, s_loc], mm_dt)
            if plan_b:
                g_out = dram.tile([4 * G_ROWS, 512], mm_dt)
            else:
                g_out = g_in

            def g_read(row0, nrows, sqc):
                """AP of gathered rows [row0:row0+nrows] for seq cols
                [512*sqc : 512*(sqc+1)]."""
                if plan_b:
                    return g_out[sqc * G_ROWS + row0: sqc * G_ROWS + row0 + nrows, :]
                return g_out[row0: row0 + nrows, sqc * 512:(sqc + 1) * 512]

            # ---- stage 1: shared projections on the local S slice ----
            def shared_proj(w_dram, w_cols, m_chunks, norm_mchunks, inv_n, row_base,
                            rope_chunk):
                """Project xt with w (feature-major out), rms-normalize the
                first norm_mchunks chunks, rope-combine rope_chunk, write to
                g_in rows starting at row_base."""
                for nch in range(nch_n):
                    raw = []   # bf16 SBUF copies of the raw projection chunks
                    ssq = psums.tile([1, 512], F32, tag="p_sum")
                    # m-groups of <=6 PSUM accumulators (4 from ppool + 2 borrowed)
                    mgs = [list(range(g, min(g + 6, m_chunks))) for g in range(0, m_chunks, 6)]
                    for mg in mgs:
                        accs = {m: (ppool.tile([128, 512], F32, tag="p_a", name="acc_s1")
                                    if j < 4 else
                                    pscore.tile([128, 512], F32, tag="p_sc", name="acc_s1b"))
                                for j, m in enumerate(mg)}
                        for k in range(HID // 128):
                            wt = wpool.tile([128, 128 * len(mg)], mm_dt, tag="w_s1")
                            nc.sync.dma_start(
                                out=wt,
                                in_=w_dram.ap()[k * 128:(k + 1) * 128,
                                                mg[0] * 128: mg[0] * 128 + 128 * len(mg)])
                            xtt = xpool.tile([128, 512], mm_dt, tag="xt_s")
                            nc.sync.dma_start(
                                out=xtt,
                                in_=xt.ap()[k * 128:(k + 1) * 128,
                                            nch * 512:(nch + 1) * 512])
                            for j, m in enumerate(mg):
                                nc.tensor.matmul(
                                    accs[m], wt[:, j * 128:(j + 1) * 128],
                                    xtt,
                                    start=(k == 0), stop=(k == HID // 128 - 1))
                        for m in mg:
                            if m < norm_mchunks:
                                sq = spool.tile([128, 512], F32, tag="sq")
                                nc.scalar.activation(out=sq, in_=accs[m],
                                                     func=mybir.ActivationFunctionType.Square)
                                # ones-reduction over partitions via matmul
                                nc.tensor.matmul(ssq, ones_f32_sb, sq,
                                                 start=(m == 0), stop=(m == norm_mchunks - 1),
                                                 skip_group_check=True)
                                r = persist.tile([128, 512], mm_dt, tag="raw%d" % m)
                                nc.vector.tensor_copy(r, accs[m])
                                raw.append((m, r))
                            elif m == rope_chunk:
                                t0 = spool.tile([D_ROPE, 512], F32, tag="ropet0")
                                t1 = spool.tile([D_ROPE, 512], F32, tag="ropet1")
                                nc.vector.tensor_tensor(
                                    t0, accs[m][0:D_ROPE, :],
                                    cosl_sb[:, nch * 512:(nch + 1) * 512],
                                    mybir.AluOpType.mult)
                                nc.vector.tensor_tensor(
                                    t1, accs[m][D_ROPE:2 * D_ROPE, :],
                                    sinl_sb[:, nch * 512:(nch + 1) * 512],
                                    mybir.AluOpType.mult)
                                pe = spool.tile([D_ROPE, 512], mm_dt, tag="ropeo")
                                nc.vector.tensor_tensor(pe, t0, t1, mybir.AluOpType.add)
                                nc.sync.dma_start(
                                    out=g_in[row_base + norm_mchunks * 128:
                                             row_base + norm_mchunks * 128 + D_ROPE,
                                             nch * 512:(nch + 1) * 512],
                                    in_=pe)
                    # rstd = 1/sqrt(mean + eps)
                    sd = spool.tile([1, 512], F32, tag="sd")
                    nc.scalar.activation(out=sd, in_=ssq,
                                         func=mybir.ActivationFunctionType.Sqrt,
                                         bias=eps_sb, scale=inv_n)
                    rstd = spool.tile([1, 512], F32, tag="rstd")
                    nc.vector.reciprocal(rstd, sd)
                    rstd_bc = spool.tile([128, 512], F32, tag="rstd_bc")
                    nc.gpsimd.partition_broadcast(rstd_bc, rstd)
                    for m, r in raw:
                        o = spool.tile([128, 512], mm_dt, tag="normo")
                        nc.vector.tensor_tensor(o, r, rstd_bc, mybir.AluOpType.mult)
                        nc.sync.dma_start(
                            out=g_in[row_base + m * 128: row_base + (m + 1) * 128,
                                     nch * 512:(nch + 1) * 512],
                            in_=o)

            ones_f32_sb = persist.tile([128, 1], F32, tag="ones_f32")
            nc.vector.memset(ones_f32_sb, 1.0)

            shared_proj(wdq, Q_LORA, Q_LORA // 128, Q_LORA // 128, 1.0 / Q_LORA, 0, None)
            shared_proj(wkva, KV_LORA + 2 * D_ROPE, 5, KV_LORA // 128, 1.0 / KV_LORA,
                        Q_LORA, 4)

            # ---- stage 1.5: AllGather within batch groups ----
            if plan_b:
                nc.gpsimd.collective_compute(
                    "AllGather", mybir.AluOpType.bypass,
                    replica_groups=GROUPS,
                    ins=[g_in.opt()], outs=[g_out.opt()])

            # ---- stage 2a: decompress KV ----
            ckv_sb = persist.tile([128, KV_LORA // 128, 4, 512], mm_dt, tag="ckv")
            for k in range(KV_LORA // 128):
                for sqc in range(4):
                    nc.sync.dma_start(out=ckv_sb[:, k, sqc, :],
                                      in_=g_read(Q_LORA + k * 128, 128, sqc))
            kpe_sb = persist.tile([D_ROPE, 4, 512], mm_dt, tag="kpe")
            for sqc in range(4):
                nc.sync.dma_start(out=kpe_sb[:, sqc, :],
                                  in_=g_read(Q_LORA + KV_LORA, D_ROPE, sqc))

            wkvb_sb = persist.tile([128, KV_LORA // 128, HPC, 256], mm_dt, tag="wkvb")
            nc.sync.dma_start(out=wkvb_sb,
                              in_=wkvb.ap().rearrange("(kc p) h c -> p kc h c", p=128))

            kn_sb = persist.tile([D_NOPE, HPC, 4, 512], mm_dt, tag="kn")
            for h in range(HPC):
                for skc in range(4):
                    acc = ppool.tile([128, 512], F32, tag="p_a")
                    for k in range(KV_LORA // 128):
                        nc.tensor.matmul(acc, wkvb_sb[:, k, h, 0:128],
                                         ckv_sb[:, k, skc, :],
                                         start=(k == 0), stop=(k == KV_LORA // 128 - 1))
                    nc.scalar.copy(kn_sb[:, h, skc, :], acc)

            v_sb = persist.tile([128, S // 128, HPC * D_V], mm_dt, tag="v")
            for skt in range(S // 128):
                acc = ppool.tile([128, 512], F32, tag="p_a")
                for k in range(KV_LORA // 128):
                    nc.tensor.matmul(
                        acc,
                        ckv_sb[:, k, skt // 4, (skt % 4) * 128:(skt % 4) * 128 + 128],
                        wkvb_sb[:, k, :, 128:256],
                        start=(k == 0), stop=(k == KV_LORA // 128 - 1))
                nc.scalar.copy(v_sb[:, skt, :], acc)

            # ---- stage 2b/2c: per-seq-chunk q up-proj + attention ----
            ctx_sb = persist.tile([D_V, HPC, 4, 512], mm_dt, tag="ctx")

            for sqc in range(4):
                qn_t = {}
                qpe_t = {}
                # stream q_norm chunks for this seq chunk
                qnorm_t = [spool.tile([128, 512], mm_dt, tag="qn_stream%d" % (k % 4),
                                      name="qnorm_t", bufs=3) for k in range(Q_LORA // 128)]
                for k in range(Q_LORA // 128):
                    nc.sync.dma_start(out=qnorm_t[k], in_=g_read(k * 128, 128, sqc))
                for g2 in range(2):   # two pairs of heads -> 4 psum banks each
                    accs = [ppool.tile([128, 512], F32, tag="p_a", name="acc_qup") for _ in range(4)]
                    for k in range(Q_LORA // 128):
                        wt = wpool.tile([128, 512], mm_dt, tag="w_uq")
                        nc.sync.dma_start(
                            out=wt,
                            in_=wuq.ap()[k * 128:(k + 1) * 128,
                                         g2 * 512:(g2 + 1) * 512])
                        for j in range(4):
                            nc.tensor.matmul(accs[j], wt[:, j * 128:(j + 1) * 128],
                                             qnorm_t[k],
                                             start=(k == 0), stop=(k == Q_LORA // 128 - 1))
                    for j in range(4):
                        h = g2 * 2 + j // 2
                        if j % 2 == 0:   # nope chunk
                            qt = spool.tile([D_NOPE, 512], mm_dt, tag="qn_h%d" % h, bufs=1)
                            nc.scalar.copy(qt, accs[j])
                            qn_t[h] = qt
                        else:            # rope chunk [E(64) | R(64)]
                            t0 = spool.tile([D_ROPE, 512], F32, tag="qropet0")
                            t1 = spool.tile([D_ROPE, 512], F32, tag="qropet1")
                            nc.vector.tensor_tensor(t0, accs[j][0:D_ROPE, :],
                                                    cosf_sb[:, sqc, :], mybir.AluOpType.mult)
                            nc.vector.tensor_tensor(t1, accs[j][D_ROPE:2 * D_ROPE, :],
                                                    sinf_sb[:, sqc, :], mybir.AluOpType.mult)
                            qt = spool.tile([D_ROPE, 512], mm_dt, tag="qpe_h%d" % h, bufs=1)
                            nc.vector.tensor_tensor(qt, t0, t1, mybir.AluOpType.add)
                            qpe_t[h] = qt

                n_skt = 4 * (sqc + 1)
                for h in range(HPC):
                    ctx_acc = pctx.tile([D_V, 512], F32, tag="p_ctx")
                    sum_acc = psums.tile([1, 512], F32, tag="p_sum")
                    for skt in range(n_skt):
                        sc = pscore.tile([128, 512], F32, tag="p_sc")
                        nc.tensor.matmul(
                            sc, kn_sb[:, h, skt // 4, (skt % 4) * 128:(skt % 4) * 128 + 128],
                            qn_t[h], start=True, stop=False)
                        nc.tensor.matmul(
                            sc, kpe_sb[:, skt // 4, (skt % 4) * 128:(skt % 4) * 128 + 128],
                            qpe_t[h], start=False, stop=True)
                        ex = spool.tile([128, 512], mm_dt, tag="exp%d" % (skt % 3), bufs=2)
                        nc.scalar.activation(out=ex, in_=sc,
                                             func=mybir.ActivationFunctionType.Exp,
                                             scale=SCALE)
                        if skt >= 4 * sqc:   # diagonal block: causal mask
                            nc.vector.tensor_tensor(ex, ex, mask_sb[:, skt - 4 * sqc, :],
                                                    mybir.AluOpType.mult)
                        nc.tensor.matmul(sum_acc, ones_sb, ex,
                                         start=(skt == 0), stop=(skt == n_skt - 1),
                                         skip_group_check=True)
                        nc.tensor.matmul(ctx_acc,
                                         v_sb[:, skt, h * D_V:(h + 1) * D_V], ex,
                                         start=(skt == 0), stop=(skt == n_skt - 1),
                                         skip_group_check=True)
                    recip = spool.tile([1, 512], F32, tag="recip")
                    nc.vector.reciprocal(recip, sum_acc)
                    recip_bc = spool.tile([128, 512], F32, tag="recip_bc")
                    nc.gpsimd.partition_broadcast(recip_bc, recip)
                    nc.vector.tensor_tensor(ctx_sb[:, h, sqc, :], ctx_acc, recip_bc,
                                            mybir.AluOpType.mult)

            # ---- stage 2d: output projection (partial sums over local heads) ----
            ow_sb = persist.tile([D_V, HPC, HID], mm_dt, tag="ow")
            nc.sync.dma_start(out=ow_sb, in_=ow.ap().rearrange("h p c -> p h c"))
            for hidc in range(HID // 128):
                for sqc in range(4):
                    acc = ppool.tile([128, 512], F32, tag="p_a")
                    for h in range(HPC):
                        nc.tensor.matmul(acc, ow_sb[:, h, hidc * 128:(hidc + 1) * 128],
                                         ctx_sb[:, h, sqc, :],
                                         start=(h == 0), stop=(h == HPC - 1))
                    o = spool.tile([128, 512], F32, tag="oout")
                    nc.scalar.copy(o, acc)
                    nc.sync.dma_start(
                        out=out_t.ap()[hidc * 128:(hidc + 1) * 128,
                                       sqc * 512:(sqc + 1) * 512],
                        in_=o)

    nc.compile()
    return nc


# ------------------------------------------------------------- host side --
def _rope_tables():
    inv_freq = 1.0 / (ROPE_THETA ** (np.arange(0, D_ROPE, 2, dtype=np.float64) / D_ROPE))
    t = np.arange(S, dtype=np.float64)
    freqs = np.outer(t, inv_freq)                    # [S, 32]
    emb = np.concatenate([freqs, freqs], axis=-1)    # [S, 64]
    return (np.cos(emb).astype(np.float32).T.copy(),
            np.sin(emb).astype(np.float32).T.copy())  # [64, S]


_E_PERM = np.concatenate([np.arange(0, D_ROPE, 2), np.arange(1, D_ROPE, 2)])


def _rope_expand(Wpe):
    """[n, 64] rope weight cols -> [n, 128]: [even/odd-reordered | rot-half signed]."""
    Y = Wpe[:, _E_PERM]
    R = np.concatenate([-Y[:, D_ROPE // 2:], Y[:, :D_ROPE // 2]], axis=1)
    return np.concatenate([Y, R], axis=1)


def _prep_inputs(hidden_states, w_dq, q_a_ln_w, w_uq, kv_a_w, kv_a_ln_w, kv_b_w, o_w,
                 plan_b=PLAN_B):
    bf = ml_dtypes.bfloat16
    s_loc = S // 4 if plan_b else S
    cosT, sinT = _rope_tables()

    wuq_eff = (np.asarray(q_a_ln_w)[:, None] * np.asarray(w_uq)).reshape(Q_LORA, H, D_Q)
    head_blocks = []
    for h in range(H):
        head_blocks.append(np.concatenate(
            [wuq_eff[:, h, :D_NOPE], _rope_expand(wuq_eff[:, h, D_NOPE:])], axis=1))
    wuq_x = np.stack(head_blocks, axis=1)            # [1536, 16, 256]

    kv_a = np.asarray(kv_a_w)
    wkva_x = np.concatenate([kv_a[:, :KV_LORA], _rope_expand(kv_a[:, KV_LORA:])],
                            axis=1).astype(bf)       # [2048, 640]
    wkvb_eff = (np.asarray(kv_a_ln_w)[:, None] * np.asarray(kv_b_w)).reshape(KV_LORA, H, 256)
    ow_r = np.asarray(o_w).reshape(H, D_V, HID)

    d = np.arange(512)[None, :]
    r = np.arange(128)[:, None]
    masks = np.stack([(d >= 128 * dd + r) for dd in range(4)]).astype(bf)  # [4,128,512]

    wdq_b = np.asarray(w_dq).astype(bf)
    hs = np.asarray(hidden_states)

    in_maps = []
    for c in range(N_CORES):
        b, hg = c // 4, c % 4
        s0 = 512 * hg if plan_b else 0
        xt = np.ascontiguousarray(hs[b].T[:, s0:s0 + s_loc]).astype(bf)
        wuq_core = np.ascontiguousarray(
            wuq_x[:, HPC * hg: HPC * (hg + 1), :].reshape(Q_LORA, HPC * 256)).astype(bf)
        in_maps.append({
            "xt": xt,
            "wdq": wdq_b,
            "wuq": wuq_core,
            "wkva": wkva_x,
            "wkvb": np.ascontiguousarray(
                wkvb_eff[:, HPC * hg: HPC * (hg + 1), :]).astype(bf),
            "ow": np.ascontiguousarray(ow_r[HPC * hg: HPC * (hg + 1)]).astype(bf),
            "cos_l": np.ascontiguousarray(cosT[:, s0:s0 + s_loc]),
            "sin_l": np.ascontiguousarray(sinT[:, s0:s0 + s_loc]),
            "cos_f": cosT.astype(bf),
            "sin_f": sinT.astype(bf),
            "masks": masks,
        })
    return in_maps


def _postprocess(results):
    out = np.empty((B, S, HID), dtype=np.float32)
    for b in range(B):
        acc = results[4 * b]["out_t"].astype(np.float32).copy()
        for c in GROUPS[b][1:]:
            acc += results[c]["out_t"]
        out[b] = acc.T
    return out


def kernel(**inputs):
    key = (PLAN_B, str(MM_DT))
    if key not in _CACHE:
        _CACHE[key] = build_kernel(PLAN_B, MM_DT)
    nc = _CACHE[key]
    in_maps = _prep_inputs(**inputs, plan_b=PLAN_B)
    r = run_bass_kernel_spmd(nc, in_maps, core_ids=list(range(N_CORES)))
    return _postprocess(r.results)
